# revision 1
# baseline (speedup 1.0000x reference)
"""Trainium2 Bass kernel for nn_BasicBlock (conv-SE-prune-BN residual block).

Data-parallel over batch across 8 NeuronCores. Per core (B_loc = 1024),
processed in groups of 3 six-sample conv tiles (18 samples):
  P0  : stream x, per-(channel,sample) spatial mean (pooling)
  fc  : fc1-relu-fc2-sigmoid gates (tiny PE matmuls)
  AG  : AllGather all B*C gates; global-threshold bisection (22 fixed
        count-below-T iterations on ACT, interleaved with conv1)
  conv1: 3x3 conv = 3 K=128/M=128 bf16 matmuls per tile; K halves are
        channels + a flat-shifted duplicate (one contiguous SBUF DMA);
        M halves are two accumulators, combined via an SBUF bounce
  P3a : out1 * relu(gate - T), BN1 partial stats
  AR2 : AllReduce BN1 stats -> affine coefs
  P3b : bn1-affine+relu -> conv2 -> BN2 partial stats
  AR3 : AllReduce BN2 stats
  P5  : bn2-affine + residual + relu -> out

kernel(**inputs) takes the FULL inputs and returns the FULL output.
"""
import numpy as np

import concourse.bacc as bacc
import concourse.bass as bass
import concourse.mybir as mybir
import concourse.tile as tile

F32 = mybir.dt.float32
BF16 = mybir.dt.bfloat16
I32 = mybir.dt.int32
AF = mybir.ActivationFunctionType
ALU = mybir.AluOpType
AX = mybir.AxisListType

C = 64
HW = 64          # 8*8 spatial
TILE_B = 6
GROUP_T = 3      # conv tiles per group (shared psum tensor / DMAs)
PRUNE_RATE = 0.2
EPS = 1e-5
BISECT_ITERS = 22
PADSZ = 10 * 9   # padded sample size


def _groups(b_loc):
    """[(b0, [nb per tile])]; all but possibly the last have full tiles."""
    tiles = []
    b0 = 0
    while b0 < b_loc:
        nb = min(TILE_B, b_loc - b0)
        tiles.append((b0, nb))
        b0 += nb
    out = []
    i = 0
    while i < len(tiles):
        grp = tiles[i:i + GROUP_T]
        out.append((grp[0][0], [nb for (_, nb) in grp]))
        i += GROUP_T
    return out


def _transpose64(nc, dst_ap, src_ap):
    # full 64x64 transpose from per-32-block vector.transpose
    for i in (0, 32):
        for j in (0, 32):
            nc.vector.transpose(out=dst_ap[j:j + 32, i:i + 32],
                                in_=src_ap[i:i + 32, j:j + 32])


def build_nc(n_cores, b_loc):
    B_glob = n_cores * b_loc
    k_prune = int(PRUNE_RATE * B_glob * C)
    G = (b_loc * C * n_cores) // 128
    # sum of sign(T-g) = 2*count_less - N ; count_less <= k <=> sum <= 2k-N
    D0s = float(2 * k_prune - B_glob * C)
    N1 = float(B_glob * HW)
    groups = _groups(b_loc)
    NG = len(groups)
    rg = [list(range(n_cores))]
    GB = GROUP_T * TILE_B

    nc = bacc.Bacc("TRN2", target_bir_lowering=False, debug=False,
                   enable_asserts=True, num_devices=n_cores)

    x_in = nc.dram_tensor("x", [b_loc, C, 8, 8], F32, kind="ExternalInput")
    w1_in = nc.dram_tensor("conv1_w", [C, C, 3, 3], F32, kind="ExternalInput")
    w2_in = nc.dram_tensor("conv2_w", [C, C, 3, 3], F32, kind="ExternalInput")
    fc1w_in = nc.dram_tensor("fc1_w", [16, C], F32, kind="ExternalInput")
    fc1b_in = nc.dram_tensor("fc1_b", [16], F32, kind="ExternalInput")
    fc2w_in = nc.dram_tensor("fc2_w", [C, 16], F32, kind="ExternalInput")
    fc2b_in = nc.dram_tensor("fc2_b", [C], F32, kind="ExternalInput")
    bn1g_in = nc.dram_tensor("bn1_g", [C], F32, kind="ExternalInput")
    bn1b_in = nc.dram_tensor("bn1_b", [C], F32, kind="ExternalInput")
    bn2g_in = nc.dram_tensor("bn2_g", [C], F32, kind="ExternalInput")
    bn2b_in = nc.dram_tensor("bn2_b", [C], F32, kind="ExternalInput")
    out_d = nc.dram_tensor("out", [b_loc, C, 8, 8], F32, kind="ExternalOutput")

    with tile.TileContext(nc) as tc:
        with (
            tc.tile_pool(name="persist", bufs=1) as pp,
            tc.tile_pool(name="stg", bufs=2) as stgp,
            tc.tile_pool(name="pads", bufs=1) as padp,
            tc.tile_pool(name="small", bufs=2) as smallp,
            tc.tile_pool(name="prer", bufs=2) as prep,
            tc.tile_pool(name="dram", bufs=1, space="DRAM") as dramp,
        ):
            # early dummy collective absorbs cross-core start skew
            bar_sb = pp.tile([1, 1], F32, tag="bar_sb")
            bar_in = dramp.tile([1, 1], F32, tag="bar_in")
            bar_out = dramp.tile([1, 1], F32, tag="bar_out")
            nc.vector.memset(bar_sb[:], 0)
            nc.sync.dma_start(bar_in[:], bar_sb[:])
            nc.gpsimd.collective_compute(
                "AllReduce", ALU.add, replica_groups=rg,
                ins=[bar_in.opt()], outs=[bar_out.opt()])

            # ---------------- constants / weights prep ----------------
            w1_sb = pp.tile([C, C, 3, 3], F32, tag="w1")
            w2_sb = pp.tile([C, C, 3, 3], F32, tag="w2")
            nc.sync.dma_start(w1_sb[:], w1_in[:])
            nc.sync.dma_start(w2_sb[:], w2_in[:])
            lhs1, lhs2 = [], []
            for (wsb, lst, nm) in ((w1_sb, lhs1, "l1"), (w2_sb, lhs2, "l2")):
                for dy in range(3):
                    lt = pp.tile([128, 128], BF16, tag=f"{nm}_{dy}")
                    nc.vector.memset(lt[:], 0)
                    for (kp, mp, dx) in ((0, 0, 0), (64, 0, 1), (64, 64, 2)):
                        tp = smallp.tile([C, C], F32, tag="wtr")
                        _transpose64(nc, tp[:], wsb[:, :, dy, dx])
                        nc.vector.tensor_copy(lt[kp:kp + 64, mp:mp + 64], tp[:])
                    lst.append(lt)

            fc1T = pp.tile([C, C], F32, tag="fc1T")   # [64, 16] used
            fc2T = pp.tile([C, C], F32, tag="fc2T")   # [16, 64] used
            for (w_in_, shape, dstT) in ((fc1w_in, (16, C), fc1T),
                                         (fc2w_in, (C, 16), fc2T)):
                tmp = smallp.tile([C, C], F32, tag="fctmp")
                nc.vector.memset(tmp[:], 0)
                nc.sync.dma_start(tmp[0:shape[0], 0:shape[1]], w_in_[:])
                _transpose64(nc, dstT[:], tmp[:])

            vecs = pp.tile([C, 8], F32, tag="vecs")
            # cols: 0=fc2_b 1=bn1_g 2=bn1_b 3=bn2_g 4=bn2_b
            nc.sync.dma_start(vecs[:, 0:1], fc2b_in[:].unsqueeze(1))
            nc.sync.dma_start(vecs[:, 1:2], bn1g_in[:].unsqueeze(1))
            nc.sync.dma_start(vecs[:, 2:3], bn1b_in[:].unsqueeze(1))
            nc.sync.dma_start(vecs[:, 3:4], bn2g_in[:].unsqueeze(1))
            nc.sync.dma_start(vecs[:, 4:5], bn2b_in[:].unsqueeze(1))
            fc1b = pp.tile([16, 1], F32, tag="fc1b")
            nc.sync.dma_start(fc1b[:], fc1b_in[:].unsqueeze(1))

            onesKM = pp.tile([128, 128], BF16, tag="ones")
            nc.vector.memset(onesKM[:], 1.0)
            eps_t = pp.tile([C, 1], F32, tag="eps")
            nc.vector.memset(eps_t[:], EPS)

            # padded-input ring [128, slot, GB, 10, 9]; borders stay 0.
            NSLOT = 2
            xpad = padp.tile([128, NSLOT, GB, 10, 9], BF16, tag="xpad")
            nc.vector.memset(xpad[:], 0)
            xpad_f = xpad[:].rearrange("p s b r w -> p s (b r w)")

            NT = sum(len(nbs) for (_, nbs) in groups)
            R = pp.tile([C, NT, TILE_B, HW], BF16, tag="R")
            pooled = pp.tile([C, b_loc], F32, tag="pooled")
            gates = pp.tile([C, b_loc], F32, tag="gates")
            # stats sections of NT per-tile cols: S1, Q1, S2, Q2 (merged
            # group ops write their sum into the group's first tile column)
            stats = pp.tile([C, 4 * NT], F32, tag="stats")
            nc.vector.memset(stats[:], 0)
            sq_l = pp.tile([C, 4], F32, tag="sq_l")
            cf1 = pp.tile([C, 2], F32, tag="cf1")
            cf2 = pp.tile([C, 2], F32, tag="cf2")
            scratch = pp.tile([C, 8], F32, tag="scratch")

            # dram bounce buffers for collectives
            ag_in = dramp.tile([C, b_loc], F32, tag="ag_in")
            ag_out = dramp.tile([n_cores, C, b_loc], F32, tag="ag_out")
            ar_in = dramp.tile([C, 2], F32, tag="ar_in")
            ar_out = dramp.tile([C, 2], F32, tag="ar_out")
            ar2_in = dramp.tile([C, 2], F32, tag="ar2_in")
            ar2_out = dramp.tile([C, 2], F32, tag="ar2_out")

            def x_src(b0, ns):
                return x_in[b0:b0 + ns].transpose([1, 0, 2, 3])

            # ---------------- P0: pooling pass ----------------
            # bigger chunks than the conv groups: the gates buffers are not
            # allocated yet, so borrow that SBUF for 36-sample staging
            p0_cm = tc.tile_pool(name="p0stg", bufs=2)
            p0p = p0_cm.__enter__()
            P0C = 2 * GB
            b0 = 0
            while b0 < b_loc:
                ns = min(P0C, b_loc - b0)
                stg = p0p.tile([C, P0C, 8, 8], F32, tag="stg0")
                nc.sync.dma_start(stg[:, 0:ns], x_src(b0, ns))
                nc.vector.tensor_reduce(out=pooled[:, b0:b0 + ns],
                                        in_=stg[:, 0:ns], axis=AX.XY,
                                        op=ALU.add)
                b0 += ns
            p0_cm.__exit__(None, None, None)

            gatap_cm = tc.tile_pool(name="gatap", bufs=1)
            gatap = gatap_cm.__enter__()
            gata = gatap.tile([128, G], F32, tag="gata")
            cjunk = gatap.tile([128, G], BF16, tag="cjunk")

            # ---------- gates: fc1 relu fc2 sigmoid (scoped psum) ----------
            with tc.tile_pool(name="ps_fc", bufs=2, space="PSUM") as psm:
                # z1 is overlaid on pooled[0:16] (each chunk read before write)
                for j in range(0, b_loc, 512):
                    e = min(j + 512, b_loc)
                    zp = psm.tile([C, 512], F32, tag="zfc")
                    nc.tensor.matmul(zp[0:16, 0:e - j], fc1T[:, 0:16],
                                     pooled[:, j:e], start=True, stop=True)
                    nc.scalar.activation(pooled[0:16, j:e], zp[0:16, 0:e - j],
                                         AF.Relu, scale=1.0 / HW, bias=fc1b[:])
                for j in range(0, b_loc, 512):
                    e = min(j + 512, b_loc)
                    zp = psm.tile([C, 512], F32, tag="zfc")
                    nc.tensor.matmul(zp[:, 0:e - j], fc2T[0:16, :],
                                     pooled[0:16, j:e], start=True, stop=True)
                    nc.scalar.activation(gates[:, j:e], zp[:, 0:e - j],
                                         AF.Sigmoid, bias=vecs[:, 0:1])

            # allgather gates, load as [128, G]
            nc.sync.dma_start(ag_in[:], gates[:])
            nc.gpsimd.collective_compute(
                "AllGather", ALU.bypass, replica_groups=rg,
                ins=[ag_in.opt()], outs=[ag_out.opt()])
            nc.sync.dma_start(
                gata[:], ag_out[:].rearrange("n c b -> (n c b)")
                .rearrange("(p g) -> p g", p=128))

            psc_cm = tc.tile_pool(name="ps_conv", bufs=2, space="PSUM")
            psc = psc_cm.__enter__()
            psb_cm = tc.tile_pool(name="ps_bis", bufs=2, space="PSUM")
            psb = psb_cm.__enter__()

            # ---------------- bisection machinery ----------------
            lh = pp.tile([128, 2], F32, tag="lh")
            Tt = pp.tile([128, 1], F32, tag="Tt")
            nc.vector.memset(lh[:, 0:1], 0.0)
            nc.vector.memset(lh[:, 1:2], 1.0)

            bis_at = {}
            bstart = NG - 2 - 2 * (BISECT_ITERS - 1)
            if bstart >= 1:
                for j in range(BISECT_ITERS):
                    bis_at[bstart + 2 * j] = 1
            else:
                bis_at[max(0, NG - 2)] = BISECT_ITERS

            def bisect_iter():
                tj = smallp.tile([128, 2], F32, tag="bj")
                nc.vector.tensor_scalar(out=tj[:], in0=lh[:], scalar1=0.5,
                                        scalar2=None, op0=ALU.mult,
                                        op1=ALU.add, accum_out=Tt[:])
                cnt = smallp.tile([128, 1], F32, tag="bcnt")
                nc.scalar.activation(cjunk[:], gata[:], AF.Sign,
                                     scale=-1.0, bias=Tt[:], accum_out=cnt[:])
                cntb = smallp.tile([128, 1], BF16, tag="bcntb")
                nc.vector.tensor_copy(cntb[:], cnt[:])
                psum_c = psb.tile([128, 1], F32, tag="bps")
                nc.tensor.matmul(psum_c[:], onesKM[:], cntb[:],
                                 start=True, stop=True)
                m_le = smallp.tile([128, 1], I32, tag="bmle")
                m_gt = smallp.tile([128, 1], I32, tag="bmgt")
                nc.vector.tensor_scalar(out=m_le[:], in0=psum_c[:],
                                        scalar1=D0s, scalar2=None,
                                        op0=ALU.is_le)
                nc.vector.tensor_scalar(out=m_gt[:], in0=psum_c[:],
                                        scalar1=D0s, scalar2=None,
                                        op0=ALU.is_gt)
                nc.vector.copy_predicated(out=lh[:, 0:1], mask=m_le[:],
                                          data=Tt[:])
                nc.vector.copy_predicated(out=lh[:, 1:2], mask=m_gt[:],
                                          data=Tt[:])

            def ps_a_view(ps):
                # A-half [64, t, b, 8, 0:8] view of grouped psum (full groups)
                return ps[0:64, :].rearrange(
                    "p (t x) -> p t x", t=GROUP_T, x=512)[:, :, 0:432] \
                    .rearrange("p t (b r w) -> p t b r w",
                               b=TILE_B, r=8, w=9)[:, :, :, :, 0:8]

            def ps_b_view(ps):
                return ps[64:128, :].rearrange(
                    "p (t x) -> p t x", t=GROUP_T, x=512)[:, :, 0:432] \
                    .rearrange("p t (b r w) -> p t b r w",
                               b=TILE_B, r=8, w=9)[:, :, :, :, 1:9]

            def conv_group(gi, slot, nbs, lhs, pad):
                """3*GROUP_T matmuls (dy-major); B-half bounced to parts 0:64."""
                ful = all(nb == TILE_B for nb in nbs)
                ps = psc.tile([128, GROUP_T * 512], F32, tag="cps")
                for dy in range(3):
                    for (t, nb) in enumerate(nbs):
                        nc.tensor.matmul(
                            ps[:, 512 * t:512 * t + nb * 72].rearrange(
                                "p (b r w) -> p b r w", b=nb, r=8, w=9),
                            lhs[dy][:],
                            pad[:, slot, TILE_B * t:TILE_B * t + nb,
                                dy:dy + 8, :],
                            start=(dy == 0), stop=(dy == 2))
                cmb = prep.tile([128, GROUP_T, TILE_B, 8, 8], BF16, tag="cmb")
                for (t, nb) in enumerate(nbs):
                    src = ps[64:128, 512 * t:512 * t + nb * 72].rearrange(
                        "p (b r w) -> p b r w", b=nb, r=8, w=9)[:, :, :, 1:9]
                    if (gi + t) % 2 == 0:
                        nc.scalar.copy(cmb[64:128, t, 0:nb], src)
                    else:
                        nc.vector.tensor_copy(cmb[64:128, t, 0:nb], src)
                if ful:
                    nc.sync.dma_start(cmb[0:64], cmb[64:128])
                else:
                    for (t, nb) in enumerate(nbs):
                        nc.sync.dma_start(cmb[0:64, t, 0:nb],
                                          cmb[64:128, t, 0:nb])
                return ps, cmb

            # ---------------- conv1 + interleaved bisection ----------------
            for (gi, (b0, nbs)) in enumerate(groups):
                slot = gi % NSLOT
                ns = sum(nbs)
                ful = all(nb == TILE_B for nb in nbs)
                stg = stgp.tile([C, GB, 8, 8], F32, tag="stg")
                nc.sync.dma_start(stg[:, 0:ns], x_src(b0, ns))
                if ful:
                    nc.scalar.activation(
                        xpad[0:64, slot, :, 1:9, 1:9], stg[:], AF.Copy)
                else:
                    st = 0
                    for (t, nb) in enumerate(nbs):
                        nc.scalar.activation(
                            xpad[0:64, slot, TILE_B * t:TILE_B * t + nb,
                                 1:9, 1:9],
                            stg[:, st:st + nb], AF.Copy)
                        st += nb
                # flat shift-by-one duplicate (single contiguous run / part)
                nc.sync.dma_start(xpad_f[64:128, slot, 0:GB * PADSZ - 1],
                                  xpad_f[0:64, slot, 1:GB * PADSZ])
                ps, cmb = conv_group(gi, slot, nbs, lhs1, xpad)
                for (t, nb) in enumerate(nbs):
                    nc.vector.tensor_tensor(
                        out=R[:, GROUP_T * gi + t, 0:nb].rearrange(
                            "p b (h w) -> p b h w", h=8, w=8),
                        in0=ps[0:64, 512 * t:512 * t + nb * 72].rearrange(
                            "p (b r w) -> p b r w", b=nb, r=8, w=9)
                        [:, :, :, 0:8],
                        in1=cmb[0:64, t, 0:nb], op=ALU.add)

                for _ in range(bis_at.get(gi, 0)):
                    bisect_iter()

            # final threshold -> -T
            tj = smallp.tile([128, 2], F32, tag="bj")
            nc.vector.tensor_scalar(out=tj[:], in0=lh[:], scalar1=0.5,
                                    scalar2=None, op0=ALU.mult,
                                    op1=ALU.add, accum_out=Tt[:])
            negT = pp.tile([128, 1], F32, tag="negT")
            nc.vector.tensor_scalar(out=negT[:], in0=Tt[:], scalar1=-1.0,
                                    scalar2=None, op0=ALU.mult)
            gatap_cm.__exit__(None, None, None)

            # ---------------- P3a: gate application + BN1 stats ----------------
            nc.scalar.activation(gates[:], gates[:], AF.Relu,
                                 bias=negT[0:64, :])
            sep = gates
            for (gi, (b0, nbs)) in enumerate(groups):
                ns = sum(nbs)
                if all(nb == TILE_B for nb in nbs):
                    rsl = R[:, GROUP_T * gi:GROUP_T * (gi + 1)].rearrange(
                        "p t b q -> p (t b) q")
                    sep_b = sep[:, b0:b0 + ns].unsqueeze(2).broadcast_to(
                        (C, ns, HW))
                    ti0 = GROUP_T * gi
                    nc.vector.scalar_tensor_tensor(
                        out=rsl, in0=rsl, scalar=1.0, in1=sep_b,
                        op0=ALU.mult, op1=ALU.mult,
                        accum_out=stats[:, ti0:ti0 + 1])
                    sqj = prep.tile([C, GB, HW], F32, tag="pre")
                    nc.scalar.activation(
                        sqj[:].rearrange("p b q -> p (b q)"),
                        rsl.rearrange("p b q -> p (b q)"), AF.Square,
                        accum_out=stats[:, NT + ti0:NT + ti0 + 1])
                else:
                    st = 0
                    for (t, nb) in enumerate(nbs):
                        ti = GROUP_T * gi + t
                        rsl = R[:, ti, 0:nb]
                        sep_b = sep[:, b0 + st:b0 + st + nb].unsqueeze(
                            2).broadcast_to((C, nb, HW))
                        nc.vector.scalar_tensor_tensor(
                            out=rsl, in0=rsl, scalar=1.0, in1=sep_b,
                            op0=ALU.mult, op1=ALU.mult,
                            accum_out=stats[:, ti:ti + 1])
                        sqj = prep.tile([C, GB, HW], F32, tag="pre")
                        nc.scalar.activation(
                            sqj[:, 0:nb].rearrange("p b q -> p (b q)"),
                            rsl.rearrange("p b q -> p (b q)"), AF.Square,
                            accum_out=stats[:, NT + ti:NT + ti + 1])
                        st += nb

            def stats_allreduce(c0, arin, arout, cf, gcol, bcol):
                nc.vector.tensor_reduce(
                    out=sq_l[:, 0:1], in_=stats[:, c0 * NT:(c0 + 1) * NT],
                    axis=AX.X, op=ALU.add)
                nc.vector.tensor_reduce(
                    out=sq_l[:, 1:2],
                    in_=stats[:, (c0 + 1) * NT:(c0 + 2) * NT],
                    axis=AX.X, op=ALU.add)
                nc.sync.dma_start(arin[:], sq_l[:, 0:2])
                nc.gpsimd.collective_compute(
                    "AllReduce", ALU.add, replica_groups=rg,
                    ins=[arin.opt()], outs=[arout.opt()])
                sq_g = smallp.tile([C, 2], F32, tag="sqg")
                nc.sync.dma_start(sq_g[:], arout[:])
                # scratch cols: 0=mean 1=E[x^2] 2=-var 3=sd 4=isd
                nc.vector.tensor_scalar(out=scratch[:, 0:2], in0=sq_g[:],
                                        scalar1=1.0 / N1, scalar2=None,
                                        op0=ALU.mult)
                nc.vector.scalar_tensor_tensor(
                    out=scratch[:, 2:3], in0=scratch[:, 0:1],
                    scalar=scratch[:, 0:1], in1=scratch[:, 1:2],
                    op0=ALU.mult, op1=ALU.subtract)
                nc.scalar.activation(scratch[:, 3:4], scratch[:, 2:3],
                                     AF.Sqrt, scale=-1.0, bias=eps_t[:])
                nc.vector.reciprocal(scratch[:, 4:5], scratch[:, 3:4])
                nc.vector.tensor_tensor(out=cf[:, 0:1],
                                        in0=vecs[:, gcol:gcol + 1],
                                        in1=scratch[:, 4:5], op=ALU.mult)
                nc.vector.scalar_tensor_tensor(
                    out=cf[:, 1:2], in0=scratch[:, 0:1],
                    scalar=cf[:, 0:1], in1=vecs[:, bcol:bcol + 1],
                    op0=ALU.mult, op1=ALU.subtract)
                nc.vector.tensor_scalar(out=cf[:, 1:2], in0=cf[:, 1:2],
                                        scalar1=-1.0, scalar2=None,
                                        op0=ALU.mult)

            stats_allreduce(0, ar_in, ar_out, cf1, 1, 2)

            # ------------- P3b: bn1+relu -> conv2 -> BN2 stats -------------
            # gates buffers are freed by now: give conv2 a 3-deep pad ring
            ypp_cm = tc.tile_pool(name="ypadp", bufs=1)
            ypp = ypp_cm.__enter__()
            YSLOT = 3
            ypad = ypp.tile([128, YSLOT, GB, 10, 9], BF16, tag="ypad")
            nc.vector.memset(ypad[:], 0)
            ypad_f = ypad[:].rearrange("p s b r w -> p s (b r w)")
            for (gi, (b0, nbs)) in enumerate(groups):
                slot = gi % YSLOT
                ful = all(nb == TILE_B for nb in nbs)
                if ful:
                    nc.scalar.activation(
                        ypad[0:64, slot, :, 1:9, 1:9],
                        R[:, GROUP_T * gi:GROUP_T * (gi + 1)].rearrange(
                            "p t b (h w) -> p (t b) h w", h=8, w=8),
                        AF.Relu, scale=cf1[:, 0:1], bias=cf1[:, 1:2])
                else:
                    for (t, nb) in enumerate(nbs):
                        nc.scalar.activation(
                            ypad[0:64, slot, TILE_B * t:TILE_B * t + nb,
                                 1:9, 1:9],
                            R[:, GROUP_T * gi + t, 0:nb].rearrange(
                                "p b (h w) -> p b h w", h=8, w=8),
                            AF.Relu, scale=cf1[:, 0:1], bias=cf1[:, 1:2])
                nc.sync.dma_start(ypad_f[64:128, slot, 0:GB * PADSZ - 1],
                                  ypad_f[0:64, slot, 1:GB * PADSZ])
                ps, cmb = conv_group(gi, slot, nbs, lhs2, ypad)
                for (t, nb) in enumerate(nbs):
                    ti = GROUP_T * gi + t
                    rsl = R[:, ti, 0:nb]
                    nc.vector.scalar_tensor_tensor(
                        out=rsl.rearrange("p b (h w) -> p b h w", h=8, w=8),
                        in0=ps[0:64, 512 * t:512 * t + nb * 72].rearrange(
                            "p (b r w) -> p b r w", b=nb, r=8, w=9)
                        [:, :, :, 0:8],
                        scalar=1.0, in1=cmb[0:64, t, 0:nb],
                        op0=ALU.mult, op1=ALU.add,
                        accum_out=stats[:, 2 * NT + ti:2 * NT + ti + 1])
                if ful:
                    ti0 = GROUP_T * gi
                    sqj = prep.tile([C, GB, HW], F32, tag="pre")
                    rfl = R[:, GROUP_T * gi:GROUP_T * (gi + 1)].rearrange(
                        "p t b q -> p (t b q)")
                    if gi % 2 == 1:
                        nc.scalar.activation(
                            sqj[:].rearrange("p b q -> p (b q)"), rfl,
                            AF.Square,
                            accum_out=stats[:, 3 * NT + ti0:3 * NT + ti0 + 1])
                    else:
                        nc.vector.scalar_tensor_tensor(
                            out=sqj[:].rearrange("p b q -> p (b q)"), in0=rfl,
                            scalar=1.0, in1=rfl, op0=ALU.mult, op1=ALU.mult,
                            accum_out=stats[:, 3 * NT + ti0:3 * NT + ti0 + 1])
                else:
                    for (t, nb) in enumerate(nbs):
                        ti = GROUP_T * gi + t
                        rsl = R[:, ti, 0:nb]
                        sqj = prep.tile([C, GB, HW], F32, tag="pre")
                        nc.scalar.activation(
                            sqj[:, 0:nb].rearrange("p b q -> p (b q)"),
                            rsl.rearrange("p b q -> p (b q)"), AF.Square,
                            accum_out=stats[:, 3 * NT + ti:3 * NT + ti + 1])

            stats_allreduce(2, ar2_in, ar2_out, cf2, 3, 4)

            # ---------------- P5: bn2 + residual + relu -> out ----------------
            latep_cm = tc.tile_pool(name="latep", bufs=3)
            latep = latep_cm.__enter__()
            for (gi, (b0, nbs)) in enumerate(groups):
                ns = sum(nbs)
                ful = all(nb == TILE_B for nb in nbs)
                stg = latep.tile([C, GB, 8, 8], F32, tag="stg5")
                nc.sync.dma_start(stg[:, 0:ns], x_src(b0, ns))
                pre = prep.tile([C, GB, HW], F32, tag="pre")
                if ful:
                    rsl = R[:, GROUP_T * gi:GROUP_T * (gi + 1)].rearrange(
                        "p t b q -> p (t b) q")
                    nc.vector.scalar_tensor_tensor(
                        out=pre[:], in0=rsl, scalar=cf2[:, 0:1],
                        in1=stg[:].rearrange("p b h w -> p b (h w)"),
                        op0=ALU.mult, op1=ALU.add)
                else:
                    st = 0
                    for (t, nb) in enumerate(nbs):
                        rsl = R[:, GROUP_T * gi + t, 0:nb]
                        nc.vector.scalar_tensor_tensor(
                            out=pre[:, st:st + nb], in0=rsl,
                            scalar=cf2[:, 0:1],
                            in1=stg[:, st:st + nb].rearrange(
                                "p b h w -> p b (h w)"),
                            op0=ALU.mult, op1=ALU.add)
                        st += nb
                nc.scalar.activation(pre[:, 0:ns], pre[:, 0:ns], AF.Relu,
                                     bias=cf2[:, 1:2])
                nc.sync.dma_start(
                    out_d[b0:b0 + ns].transpose([1, 0, 2, 3]),
                    pre[:, 0:ns].rearrange("p b (h w) -> p b h w", h=8, w=8))

            latep_cm.__exit__(None, None, None)
            ypp_cm.__exit__(None, None, None)
            psb_cm.__exit__(None, None, None)
            psc_cm.__exit__(None, None, None)

    nc.compile()
    return nc


_NC_CACHE = {}


def _get_nc(n_cores, b_loc):
    key = (n_cores, b_loc)
    if key not in _NC_CACHE:
        _NC_CACHE[key] = build_nc(n_cores, b_loc)
    return _NC_CACHE[key]


def kernel(**inputs):
    from concourse.bass_utils import run_bass_kernel_spmd

    x = np.asarray(inputs["x"], dtype=np.float32)
    B = x.shape[0]
    n_cores = 8
    b_loc = B // n_cores
    nc = _get_nc(n_cores, b_loc)

    weight_names = ["conv1_w", "conv2_w", "fc1_w", "fc1_b", "fc2_w", "fc2_b",
                    "bn1_g", "bn1_b", "bn2_g", "bn2_b"]
    in_maps = []
    for c in range(n_cores):
        m = {"x": np.ascontiguousarray(x[c * b_loc:(c + 1) * b_loc])}
        for n in weight_names:
            m[n] = np.asarray(inputs[n], dtype=np.float32)
        in_maps.append(m)
    res = run_bass_kernel_spmd(nc, in_maps, core_ids=list(range(n_cores)))
    out = np.concatenate([res.results[c]["out"] for c in range(n_cores)],
                         axis=0)
    return out.astype(np.float32)



# revision 11
# speedup vs baseline: 1.7819x; 1.7819x over previous
"""Trainium2 Bass kernel for nn_BasicBlock (conv-SE-prune-BN residual block).

Data-parallel over batch across 8 NeuronCores; b_loc = 1024 per core.

v3 design (vs baseline): single x load, everything SBUF-resident.
 - Host pre-transposes x to [C, b_loc, 8, 8] and casts to bf16; output is
   returned bf16 [C, b_loc, 8, 8] and cast back on host.
 - Samples are split into two partition halves: batch 0:512 lives on
   partitions 0:64 ("L"), batch 512:1024 on 64:128 ("H"), giving 128-wide
   elementwise ops. Conv groups alternate L/H; the conv lhs has an L and
   an H variant (output accumulator halves swapped) so conv outputs land
   on their home partitions.
 - Conv: 3 matmuls per 6-sample tile, K=128 = channels + flat-shifted
   duplicate, M=128 = two accumulators (A, B). A evacuated by ACT/DVE
   copy psum->SBUF bf16; B evacuated to a bounce buffer and merged into
   R by a gpsimd DMA with accum_op=add (CCE inline add, crosses
   partitions for free).
 - Padded layout per sample is 10 rows x 10 cols, image at rows 1:9,
   cols 2:10 (so interior rows are 4-byte aligned for DVE 2x/4x modes).
   rhs views take cols 1:10; A-half out = view cols [0:8], B = [1:9].
 - Pooling from the resident bf16 copy (pair-packed, 128 partitions),
   fc gates computed pair-packed via block-packed fc weights, AllGather,
   then threshold bisection (14 iters) on a 1/8 subsample, all
   interleaved with conv1 groups. P3a (gate apply + BN1 stats) also
   interleaves with conv1's back half.
 - BN stats: per-group accum_out columns, reduced + partition-folded
   (gpsimd DMA add) + AllReduduced; coefs duplicated to both halves.
 - P5 (bn2 affine + residual + relu) runs pair-packed from SBUF.
"""
import numpy as np

import concourse.bacc as bacc
import concourse.bass as bass
import concourse.mybir as mybir
import concourse.tile as tile

F32 = mybir.dt.float32
BF16 = mybir.dt.bfloat16
I32 = mybir.dt.int32
AF = mybir.ActivationFunctionType
ALU = mybir.AluOpType
AX = mybir.AxisListType

C = 64
HW = 64
TILE_B = 6
GB = 18          # samples per conv group
PRUNE_RATE = 0.2
EPS = 1e-5
BISECT_ITERS = 14
SUB = 64         # bisect subsample columns per (core, partition)
PR, PW = 10, 10  # padded rows / cols per sample
PADSZ = PR * PW


def _pairs(half):
    """[(j, s0, ns)] covering one 512-sample half by 18-sample groups."""
    out = []
    s0 = 0
    j = 0
    while s0 < half:
        ns = min(GB, half - s0)
        out.append((j, s0, ns))
        s0 += ns
        j += 1
    return out


def _tiles(ns):
    t, b0 = [], 0
    while b0 < ns:
        nb = min(TILE_B, ns - b0)
        t.append((b0, nb))
        b0 += nb
    return t


def _transpose64(nc, dst_ap, src_ap):
    for i in (0, 32):
        for j in (0, 32):
            nc.vector.transpose(out=dst_ap[j:j + 32, i:i + 32],
                                in_=src_ap[i:i + 32, j:j + 32])


def build_nc(n_cores, b_loc):
    B_glob = n_cores * b_loc
    HALF = b_loc // 2
    N1 = float(B_glob * HW)
    n_sub = n_cores * 128 * SUB
    k_sub = PRUNE_RATE * n_sub
    D0s = float(2.0 * k_sub - n_sub)
    rg = [list(range(n_cores))]

    pairs = _pairs(HALF)          # 29 pairs
    NP = len(pairs)
    NG = 2 * NP                   # 58 groups, order L0 H0 L1 H1 ...

    nc = bacc.Bacc("TRN2", target_bir_lowering=False, debug=False,
                   enable_asserts=True, num_devices=n_cores)

    x_in = nc.dram_tensor("x", [C, b_loc, PR, PW], BF16,
                          kind="ExternalInput")
    xr_in = nc.dram_tensor("xr", [C, b_loc, 8, 8], BF16,
                           kind="ExternalInput")
    w1_in = nc.dram_tensor("conv1_w", [C, C, 3, 3], F32, kind="ExternalInput")
    w2_in = nc.dram_tensor("conv2_w", [C, C, 3, 3], F32, kind="ExternalInput")
    fc1w_in = nc.dram_tensor("fc1_w", [16, C], F32, kind="ExternalInput")
    fc1b_in = nc.dram_tensor("fc1_b", [16], F32, kind="ExternalInput")
    fc2w_in = nc.dram_tensor("fc2_w", [C, 16], F32, kind="ExternalInput")
    fc2b_in = nc.dram_tensor("fc2_b", [C], F32, kind="ExternalInput")
    bn1g_in = nc.dram_tensor("bn1_g", [C], F32, kind="ExternalInput")
    bn1b_in = nc.dram_tensor("bn1_b", [C], F32, kind="ExternalInput")
    bn2g_in = nc.dram_tensor("bn2_g", [C], F32, kind="ExternalInput")
    bn2b_in = nc.dram_tensor("bn2_b", [C], F32, kind="ExternalInput")
    out_d = nc.dram_tensor("out", [C, b_loc, 8, 8], BF16, kind="ExternalOutput")

    with tile.TileContext(nc) as tc:
        with (
            tc.tile_pool(name="persist", bufs=1) as pp,
            tc.tile_pool(name="rings", bufs=2) as rp,
            tc.tile_pool(name="small", bufs=2) as smallp,
            tc.tile_pool(name="psc", bufs=2, space="PSUM") as psc,
            tc.tile_pool(name="psf", bufs=1, space="PSUM") as psf,
            tc.tile_pool(name="dram", bufs=1, space="DRAM") as dramp,
        ):
            # ---------------- persistent SBUF ----------------
            resid = pp.tile([128, HALF * HW], BF16, tag="resid")
            R = pp.tile([128, HALF * HW], BF16, tag="R")
            xpad = pp.tile([128, 3, GB, PR, PW], BF16, tag="xpad")
            ypad = pp.tile([128, 3, GB, PR, PW], BF16, tag="ypad")
            pooled = pp.tile([128, HALF], F32, tag="pooled")
            gates = pp.tile([128, HALF], F32, tag="gates")
            sep = pp.tile([128, HALF], BF16, tag="sep")
            gata_s = pp.tile([128, n_cores, SUB], BF16, tag="gata_s")
            onesKM = pp.tile([128, 128], BF16, tag="onesKM")
            stats = pp.tile([128, 192], F32, tag="stats")
            q2s = pp.tile([128, 32], F32, tag="q2s")
            vecs = pp.tile([128, 8], F32, tag="vecs")
            # vecs cols: 0=bn1_g 1=bn1_b 2=bn2_g 3=bn2_b
            fc1b = pp.tile([32, 1], F32, tag="fc1b")
            fc1T = pp.tile([128, 32], F32, tag="fc1T")
            fc2T = pp.tile([32, 128], F32, tag="fc2T")
            cf1 = pp.tile([128, 2], F32, tag="cf1")
            cf2 = pp.tile([128, 2], F32, tag="cf2")
            eps_t = pp.tile([C, 1], F32, tag="eps")
            lh = pp.tile([128, 2], F32, tag="lh")
            Tt = pp.tile([128, 1], F32, tag="Tt")
            negT = pp.tile([128, 1], F32, tag="negT")
            cjunk = pp.tile([128, n_cores * SUB], BF16, tag="cjunk")
            yst = pp.tile([128, 2, GB, PR, PW], BF16, tag="yst")
            scratch = pp.tile([C, 8], F32, tag="scratch")
            sqt = pp.tile([128, 4], F32, tag="sqt")

            xpad_f = xpad[:].rearrange("p s b r w -> p s (b r w)")
            ypad_f = ypad[:].rearrange("p s b r w -> p s (b r w)")

            # dram bounce buffers
            bar_sb = pp.tile([1, 1], F32, tag="bar_sb")
            bar_in = dramp.tile([1, 1], F32, tag="bar_in")
            bar_out = dramp.tile([1, 1], F32, tag="bar_out")
            ag_in = dramp.tile([128, HALF], F32, tag="ag_in")
            ag_out = dramp.tile([n_cores, 128, HALF], F32, tag="ag_out")
            ar1_in = dramp.tile([C, 2], F32, tag="ar1_in")
            ar1_out = dramp.tile([C, 2], F32, tag="ar1_out")
            ar2_in = dramp.tile([C, 2], F32, tag="ar2_in")
            ar2_out = dramp.tile([C, 2], F32, tag="ar2_out")

            # early dummy collective absorbs cross-core start skew
            nc.vector.memset(bar_sb[:], 0)
            nc.sync.dma_start(bar_in[:], bar_sb[:])
            nc.gpsimd.collective_compute(
                "AllReduce", ALU.add, replica_groups=rg,
                ins=[bar_in.opt()], outs=[bar_out.opt()])

            # ---------------- constants / weights prep ----------------
            nc.vector.memset(xpad[:], 0)
            nc.vector.memset(ypad[:], 0)
            nc.vector.memset(yst[:], 0)
            nc.vector.memset(stats[:], 0)
            nc.vector.memset(q2s[:], 0)
            nc.vector.memset(onesKM[:], 1.0)
            nc.vector.memset(eps_t[:], EPS)
            nc.vector.memset(lh[:, 0:1], 0.0)
            nc.vector.memset(lh[:, 1:2], 1.0)

            w1_sb = smallp.tile([C, C, 3, 3], F32, tag="w1")
            w2_sb = smallp.tile([C, C, 3, 3], F32, tag="w2")
            nc.sync.dma_start(w1_sb[:], w1_in[:])
            nc.sync.dma_start(w2_sb[:], w2_in[:])
            # lhs[conv][par][dy] ; L: A at M0:64, B at 64:128 ; H swapped
            lhs = [[[None] * 3 for _ in range(2)] for _ in range(2)]
            for ci, wsb in ((0, w1_sb), (1, w2_sb)):
                for dy in range(3):
                    tps = []
                    for dx in range(3):
                        tp = smallp.tile([C, C], F32, tag=f"wtr{dx}")
                        _transpose64(nc, tp[:], wsb[:, :, dy, dx])
                        tps.append(tp)
                    for par in range(2):
                        lt = pp.tile([128, 128], BF16, tag=f"l{ci}{par}{dy}")
                        nc.vector.memset(lt[:], 0)
                        ma, mb = (0, 64) if par == 0 else (64, 0)
                        nc.vector.tensor_copy(lt[0:64, ma:ma + 64], tps[0][:])
                        nc.vector.tensor_copy(lt[64:128, ma:ma + 64], tps[1][:])
                        nc.vector.tensor_copy(lt[64:128, mb:mb + 64], tps[2][:])
                        lhs[ci][par][dy] = lt

            # fc weights, block-packed for pair-packed pooled/gates
            tmp = smallp.tile([C, C], F32, tag="fctmp")
            nc.vector.memset(tmp[:], 0)
            nc.sync.dma_start(tmp[0:16, :], fc1w_in[:])
            tmpT = smallp.tile([C, C], F32, tag="fctmpT")
            _transpose64(nc, tmpT[:], tmp[:])
            nc.vector.memset(fc1T[:], 0)
            nc.vector.tensor_copy(fc1T[0:64, 0:16], tmpT[:, 0:16])
            nc.sync.dma_start(fc1T[64:128, 16:32], fc1T[0:64, 0:16])

            tmp2 = smallp.tile([C, C], F32, tag="fctmp2")
            nc.vector.memset(tmp2[:], 0)
            nc.sync.dma_start(tmp2[0:C, 0:16], fc2w_in[:])
            tmp3 = smallp.tile([C, C], F32, tag="fctmp3")
            _transpose64(nc, tmp3[:], tmp2[:])
            nc.vector.memset(fc2T[:], 0)
            nc.vector.tensor_copy(fc2T[0:16, 0:64], tmp3[0:16, :])
            nc.sync.dma_start(fc2T[16:32, 64:128], fc2T[0:16, 0:64])

            nc.sync.dma_start(fc1b[0:16, :], fc1b_in[:].unsqueeze(1))
            nc.sync.dma_start(fc1b[16:32, :], fc1b_in[:].unsqueeze(1))
            fc2b = pp.tile([128, 1], F32, tag="fc2b")
            nc.sync.dma_start(fc2b[0:64, :], fc2b_in[:].unsqueeze(1))
            nc.sync.dma_start(fc2b[64:128, :], fc2b_in[:].unsqueeze(1))
            for (col, src) in ((0, bn1g_in), (1, bn1b_in),
                               (2, bn2g_in), (3, bn2b_in)):
                nc.sync.dma_start(vecs[0:64, col:col + 1], src[:].unsqueeze(1))
                nc.sync.dma_start(vecs[64:128, col:col + 1],
                                  src[:].unsqueeze(1))

            # ---------------- group table ----------------
            # group gi = 2j + par ; par 0 = L (parts 0:64), 1 = H (64:128)
            groups = []
            for (j, s0, ns) in pairs:
                for par in range(2):
                    groups.append((j, par, s0, ns))

            # all resid loads up-front on the scalar HWDGE queue
            for (j, par, s0, ns) in groups:
                h = 64 * par
                nc.scalar.dma_start(
                    resid[h:h + 64, s0 * HW:(s0 + ns) * HW],
                    xr_in[:, HALF * par + s0:HALF * par + s0 + ns]
                    .rearrange("p b h w -> p (b h w)"))

            # ---------------- deferred-work schedule ----------------
            pool_at = {}      # group -> list of pair js
            jj = 0
            g = 2
            while jj < NP:
                take = min(2, NP - jj)
                pool_at.setdefault(g, []).extend(range(jj, jj + take))
                jj += take
                g += 1
            FC_AT = g + 1          # ~17
            bis_at = {}
            g = FC_AT + 2
            it = 0
            while it < BISECT_ITERS:
                take = min(2, BISECT_ITERS - it)
                bis_at[g] = take
                it += take
                g += 1
            SEP_AT = g             # threshold ready; make sep
            p3a_at = {}
            nxt = SEP_AT + 1
            for j in range(NP):
                gg = max(nxt, 2 * j + 2)
                if gg < NG:
                    p3a_at.setdefault(gg, []).append(j)
                    nxt = gg + 1
                # else: handled post-loop
            p3a_done = {j for v in p3a_at.values() for j in v}

            # ---------------- helper ops ----------------
            def conv_mms(ci, par, slot, ns, pad):
                ps = psc.tile([128, 3, 512], F32, tag="cps")
                tl = _tiles(ns)
                for dy in range(3):
                    for (t, (tb0, nb)) in enumerate(tl):
                        nc.tensor.matmul(
                            ps[:, t, 0:nb * 72].rearrange(
                                "p (b r w) -> p b r w", b=nb, r=8, w=9),
                            lhs[ci][par][dy][:],
                            pad[:, slot, tb0:tb0 + nb, dy:dy + 8, 1:10],
                            start=(dy == 0), stop=(dy == 2))
                return ps, tl

            def evac(ci, gi, par, s0, ns, ps, tl):
                """A->R, B->cmb, gpsimd dma-add cmb into R."""
                h = 64 * par
                ho = 64 - h
                c0 = s0 * HW
                W = ns * HW
                cmb = rp.tile([128, GB * HW], BF16, tag="cmb")
                # engine split: alternate which engine takes A vs B
                a_on_act = (gi % 2 == 0)
                full = (ns == GB)
                if full:
                    srcA = ps[h:h + 64, :, 0:432].rearrange(
                        "p t (b r w) -> p t b r w", b=6, r=8, w=9)[
                        :, :, :, :, 0:8]
                    srcB = ps[ho:ho + 64, :, 0:432].rearrange(
                        "p t (b r w) -> p t b r w", b=6, r=8, w=9)[
                        :, :, :, :, 1:9]
                    dstA = R[h:h + 64, c0:c0 + W].rearrange(
                        "p (t b r w) -> p t b r w", t=3, b=6, r=8, w=8)
                    dstB = cmb[ho:ho + 64, 0:W].rearrange(
                        "p (t b r w) -> p t b r w", t=3, b=6, r=8, w=8)
                    if ci == 0:
                        if a_on_act:
                            nc.scalar.activation(dstA, srcA, AF.Copy)
                            nc.vector.tensor_copy(dstB, srcB)
                        else:
                            nc.vector.tensor_copy(dstA, srcA)
                            nc.scalar.activation(dstB, srcB, AF.Copy)
                    else:
                        ca = stats[:, 64 + gi:65 + gi]
                        cb = stats[:, 128 + gi:129 + gi]
                        if a_on_act:
                            nc.scalar.activation(dstA, srcA, AF.Copy,
                                                 accum_out=ca[h:h + 64, :])
                            nc.vector.tensor_scalar(
                                out=dstB, in0=srcB, scalar1=1.0, scalar2=0.0,
                                op0=ALU.mult, op1=ALU.add,
                                accum_out=cb[ho:ho + 64, :])
                        else:
                            nc.vector.tensor_scalar(
                                out=dstA, in0=srcA, scalar1=1.0, scalar2=0.0,
                                op0=ALU.mult, op1=ALU.add,
                                accum_out=ca[h:h + 64, :])
                            nc.scalar.activation(dstB, srcB, AF.Copy,
                                                 accum_out=cb[ho:ho + 64, :])
                else:
                    for (t, (tb0, nb)) in enumerate(tl):
                        srcA = ps[h:h + 64, t, 0:nb * 72].rearrange(
                            "p (b r w) -> p b r w", b=nb, r=8, w=9)[
                            :, :, :, 0:8]
                        srcB = ps[ho:ho + 64, t, 0:nb * 72].rearrange(
                            "p (b r w) -> p b r w", b=nb, r=8, w=9)[
                            :, :, :, 1:9]
                        dstA = R[h:h + 64,
                                 c0 + tb0 * HW:c0 + (tb0 + nb) * HW].rearrange(
                            "p (b r w) -> p b r w", b=nb, r=8, w=8)
                        dstB = cmb[ho:ho + 64,
                                   tb0 * HW:(tb0 + nb) * HW].rearrange(
                            "p (b r w) -> p b r w", b=nb, r=8, w=8)
                        if ci == 0:
                            if t % 2 == 0:
                                nc.scalar.activation(dstA, srcA, AF.Copy)
                                nc.vector.tensor_copy(dstB, srcB)
                            else:
                                nc.vector.tensor_copy(dstA, srcA)
                                nc.scalar.activation(dstB, srcB, AF.Copy)
                        else:
                            cia = 64 + gi if t == 0 else 122 + (gi - 56)
                            cib = 128 + gi if t == 0 else 186 + (gi - 56)
                            ca = stats[:, cia:cia + 1]
                            cb = stats[:, cib:cib + 1]
                            nc.scalar.activation(dstA, srcA, AF.Copy,
                                                 accum_out=ca[h:h + 64, :])
                            nc.vector.tensor_scalar(
                                out=dstB, in0=srcB, scalar1=1.0, scalar2=0.0,
                                op0=ALU.mult, op1=ALU.add,
                                accum_out=cb[ho:ho + 64, :])
                nc.gpsimd.dma_start(R[h:h + 64, c0:c0 + W],
                                    cmb[ho:ho + 64, 0:W],
                                    accum_op=ALU.add)

            def bisect_iter():
                tj = smallp.tile([128, 2], F32, tag="bj")
                nc.vector.tensor_scalar(out=tj[:], in0=lh[:], scalar1=0.5,
                                        scalar2=None, op0=ALU.mult,
                                        op1=ALU.add, accum_out=Tt[:])
                cnt = smallp.tile([128, 1], F32, tag="bcnt")
                nc.scalar.activation(
                    cjunk[:], gata_s[:].rearrange("p n b -> p (n b)"),
                    AF.Sign, scale=-1.0, bias=Tt[:], accum_out=cnt[:])
                cntb = smallp.tile([128, 1], BF16, tag="bcntb")
                nc.vector.tensor_copy(cntb[:], cnt[:])
                psum_c = psf.tile([128, 512], F32, tag="bps")
                nc.tensor.matmul(psum_c[:, 0:1], onesKM[:], cntb[:],
                                 start=True, stop=True)
                m_le = smallp.tile([128, 1], I32, tag="bmle")
                m_gt = smallp.tile([128, 1], I32, tag="bmgt")
                nc.vector.tensor_scalar(out=m_le[:], in0=psum_c[:, 0:1],
                                        scalar1=D0s, scalar2=None,
                                        op0=ALU.is_le)
                nc.vector.tensor_scalar(out=m_gt[:], in0=psum_c[:, 0:1],
                                        scalar1=D0s, scalar2=None,
                                        op0=ALU.is_gt)
                nc.vector.copy_predicated(out=lh[:, 0:1], mask=m_le[:],
                                          data=Tt[:])
                nc.vector.copy_predicated(out=lh[:, 1:2], mask=m_gt[:],
                                          data=Tt[:])

            def p3a_pair(j, s0, ns):
                c0 = s0 * HW
                W = ns * HW
                rv = R[:, c0:c0 + W].rearrange("p (b q) -> p b q", b=ns)
                sb = sep[:, s0:s0 + ns].unsqueeze(2).broadcast_to(
                    (128, ns, HW))
                nc.vector.scalar_tensor_tensor(
                    out=rv, in0=rv, scalar=1.0, in1=sb,
                    op0=ALU.mult, op1=ALU.mult,
                    accum_out=stats[:, j:j + 1])
                sqj = rp.tile([128, GB * HW], BF16, tag="sqj")
                nc.scalar.activation(
                    sqj[:, 0:W], R[:, c0:c0 + W], AF.Square,
                    accum_out=stats[:, 32 + j:33 + j])

            def pool_pair(s0, ns):
                nc.vector.tensor_reduce(
                    out=pooled[:, s0:s0 + ns],
                    in_=resid[:, s0 * HW:(s0 + ns) * HW].rearrange(
                        "p (b q) -> p b q", b=ns),
                    axis=AX.X, op=ALU.add)

            def fc_chain():
                zp = psf.tile([128, 512], F32, tag="zfc")
                nc.tensor.matmul(zp[0:32, 0:HALF], fc1T[:],
                                 pooled[:, 0:HALF], start=True, stop=True)
                z1 = smallp.tile([32, 512], F32, tag="z1")
                nc.scalar.activation(z1[:, 0:HALF], zp[0:32, 0:HALF],
                                     AF.Relu, scale=1.0 / HW, bias=fc1b[:])
                zp2 = psf.tile([128, 512], F32, tag="zfc")
                nc.tensor.matmul(zp2[:, 0:HALF], fc2T[:],
                                 z1[:, 0:HALF], start=True, stop=True)
                nc.scalar.activation(gates[:, 0:HALF], zp2[:, 0:HALF],
                                     AF.Sigmoid, bias=fc2b[:])
                nc.sync.dma_start(ag_in[:], gates[:])
                nc.gpsimd.collective_compute(
                    "AllGather", ALU.bypass, replica_groups=rg,
                    ins=[ag_in.opt()], outs=[ag_out.opt()])
                nc.gpsimd.dma_start(
                    gata_s[:],
                    ag_out[:, :, 0:SUB].rearrange("n p b -> p n b"))

            def make_sep():
                tj = smallp.tile([128, 2], F32, tag="bj")
                nc.vector.tensor_scalar(out=tj[:], in0=lh[:], scalar1=0.5,
                                        scalar2=None, op0=ALU.mult,
                                        op1=ALU.add, accum_out=Tt[:])
                nc.vector.tensor_scalar(out=negT[:], in0=Tt[:], scalar1=-1.0,
                                        scalar2=None, op0=ALU.mult)
                nc.scalar.activation(sep[:], gates[:], AF.Relu,
                                     bias=negT[:])

            def deferred(gi):
                for j in pool_at.get(gi, []):
                    pool_pair(pairs[j][1], pairs[j][2])
                if gi == FC_AT:
                    fc_chain()
                for _ in range(bis_at.get(gi, 0)):
                    bisect_iter()
                if gi == SEP_AT:
                    make_sep()
                for j in p3a_at.get(gi, []):
                    p3a_pair(j, pairs[j][1], pairs[j][2])

            # ---------------- conv1 loop ----------------
            for (gi, (j, par, s0, ns)) in enumerate(groups):
                slot = gi % 3
                h = 64 * par
                c0 = s0 * HW
                # flat padded load straight from HBM
                nc.sync.dma_start(
                    xpad_f[0:64, slot, 0:ns * PADSZ],
                    x_in[:, HALF * par + s0:HALF * par + s0 + ns]
                    .rearrange("p b r w -> p (b r w)"))
                # flat-shift duplicate
                nc.sync.dma_start(
                    xpad_f[64:128, slot, 0:ns * PADSZ - 1],
                    xpad_f[0:64, slot, 1:ns * PADSZ])
                ps, tl = conv_mms(0, par, slot, ns, xpad)
                evac(0, gi, par, s0, ns, ps, tl)
                deferred(gi)

            # leftover deferred work
            for j in range(NP):
                if j not in p3a_done:
                    p3a_pair(j, pairs[j][1], pairs[j][2])

            # ---------------- BN1 allreduce ----------------
            def stats_ar(scol, qcol, slen, qt, arin, arout, cf, gcol, bcol):
                nc.vector.tensor_reduce(
                    out=sqt[:, 0:1], in_=stats[:, scol:scol + slen],
                    axis=AX.X, op=ALU.add)
                if qt is None:
                    nc.vector.tensor_reduce(
                        out=sqt[:, 1:2], in_=stats[:, qcol:qcol + slen],
                        axis=AX.X, op=ALU.add)
                else:
                    nc.vector.tensor_reduce(
                        out=sqt[:, 1:2], in_=qt[:], axis=AX.X, op=ALU.add)
                nc.gpsimd.dma_start(sqt[0:64, 0:2], sqt[64:128, 0:2],
                                    accum_op=ALU.add)
                nc.sync.dma_start(arin[:], sqt[0:64, 0:2])
                nc.gpsimd.collective_compute(
                    "AllReduce", ALU.add, replica_groups=rg,
                    ins=[arin.opt()], outs=[arout.opt()])
                sq_g = smallp.tile([C, 2], F32, tag="sqg")
                nc.sync.dma_start(sq_g[:], arout[:])
                # scratch cols: 0=mean 1=E[x^2] 2=-var 3=sd 4=isd
                nc.vector.tensor_scalar(out=scratch[:, 0:2], in0=sq_g[:],
                                        scalar1=1.0 / N1, scalar2=None,
                                        op0=ALU.mult)
                nc.vector.scalar_tensor_tensor(
                    out=scratch[:, 2:3], in0=scratch[:, 0:1],
                    scalar=scratch[:, 0:1], in1=scratch[:, 1:2],
                    op0=ALU.mult, op1=ALU.subtract)
                nc.scalar.activation(scratch[:, 3:4], scratch[:, 2:3],
                                     AF.Sqrt, scale=-1.0, bias=eps_t[:])
                nc.vector.reciprocal(scratch[:, 4:5], scratch[:, 3:4])
                nc.vector.tensor_tensor(out=cf[0:64, 0:1],
                                        in0=vecs[0:64, gcol:gcol + 1],
                                        in1=scratch[:, 4:5], op=ALU.mult)
                nc.vector.scalar_tensor_tensor(
                    out=cf[0:64, 1:2], in0=scratch[:, 0:1],
                    scalar=cf[0:64, 0:1], in1=vecs[0:64, bcol:bcol + 1],
                    op0=ALU.mult, op1=ALU.subtract)
                nc.vector.tensor_scalar(out=cf[0:64, 1:2], in0=cf[0:64, 1:2],
                                        scalar1=-1.0, scalar2=None,
                                        op0=ALU.mult)
                nc.sync.dma_start(cf[64:128, :], cf[0:64, :])

            stats_ar(0, 32, 32, None, ar1_in, ar1_out, cf1, 0, 1)

            # ---------------- conv2 loop ----------------
            for (gi, (j, par, s0, ns)) in enumerate(groups):
                slot = gi % 3
                h = 64 * par
                c0 = s0 * HW
                rv = R[h:h + 64, c0:c0 + ns * HW].rearrange(
                    "p (b r w) -> p b r w", b=ns, r=8, w=8)
                if par == 0:
                    nc.scalar.activation(
                        ypad[0:64, slot, 0:ns, 1:9, 2:10], rv, AF.Relu,
                        scale=cf1[0:64, 0:1], bias=cf1[0:64, 1:2])
                else:
                    ys = (gi // 2) % 2
                    nc.scalar.activation(
                        yst[64:128, ys, 0:ns, 1:9, 2:10], rv,
                        AF.Relu, scale=cf1[64:128, 0:1],
                        bias=cf1[64:128, 1:2])
                    nc.sync.dma_start(
                        ypad_f[0:64, slot, 0:ns * PADSZ],
                        yst[:].rearrange("p s b r w -> p s (b r w)")
                        [64:128, ys, 0:ns * PADSZ])
                nc.sync.dma_start(
                    ypad_f[64:128, slot, 0:ns * PADSZ - 1],
                    ypad_f[0:64, slot, 1:ns * PADSZ])
                ps, tl = conv_mms(1, par, slot, ns, ypad)
                evac(1, gi, par, s0, ns, ps, tl)
                if par == 1:
                    # Q2 over the completed pair
                    c0p = pairs[j][1] * HW
                    Wp = pairs[j][2] * HW
                    sqj = rp.tile([128, GB * HW], BF16, tag="sqj")
                    nc.vector.scalar_tensor_tensor(
                        out=sqj[:, 0:Wp], in0=R[:, c0p:c0p + Wp],
                        scalar=1.0, in1=R[:, c0p:c0p + Wp],
                        op0=ALU.mult, op1=ALU.mult,
                        accum_out=q2s[:, j:j + 1])

            stats_ar(64, 0, 124, q2s, ar2_in, ar2_out, cf2, 2, 3)

            # ---------------- P5 ----------------
            for (j, s0, ns) in pairs:
                c0 = s0 * HW
                W = ns * HW
                obuf = rp.tile([128, GB * HW], BF16, tag="obuf")
                nc.vector.scalar_tensor_tensor(
                    out=obuf[:, 0:W], in0=R[:, c0:c0 + W],
                    scalar=cf2[:, 0:1], in1=resid[:, c0:c0 + W],
                    op0=ALU.mult, op1=ALU.add)
                nc.vector.tensor_scalar(
                    out=obuf[:, 0:W], in0=obuf[:, 0:W],
                    scalar1=cf2[:, 1:2], scalar2=0.0,
                    op0=ALU.add, op1=ALU.max)
                nc.sync.dma_start(
                    out_d[:, s0:s0 + ns],
                    obuf[0:64, 0:W].rearrange("p (b h w) -> p b h w",
                                              b=ns, h=8, w=8))
                nc.sync.dma_start(
                    out_d[:, HALF + s0:HALF + s0 + ns],
                    obuf[64:128, 0:W].rearrange("p (b h w) -> p b h w",
                                                b=ns, h=8, w=8))

    nc.compile()
    return nc


_NC_CACHE = {}


def _get_nc(n_cores, b_loc):
    key = (n_cores, b_loc)
    if key not in _NC_CACHE:
        _NC_CACHE[key] = build_nc(n_cores, b_loc)
    return _NC_CACHE[key]


def make_in_maps(inputs, n_cores=8):
    import ml_dtypes

    x = np.asarray(inputs["x"], dtype=np.float32)
    b_loc = x.shape[0] // n_cores
    weight_names = ["conv1_w", "conv2_w", "fc1_w", "fc1_b", "fc2_w", "fc2_b",
                    "bn1_g", "bn1_b", "bn2_g", "bn2_b"]
    in_maps = []
    for c in range(n_cores):
        xc = x[c * b_loc:(c + 1) * b_loc].transpose(1, 0, 2, 3)
        xr = np.ascontiguousarray(xc).astype(ml_dtypes.bfloat16)
        xp = np.zeros((64, b_loc, 10, 10), dtype=ml_dtypes.bfloat16)
        xp[:, :, 1:9, 2:10] = xr
        m = {"x": xp, "xr": xr}
        for n in weight_names:
            m[n] = np.asarray(inputs[n], dtype=np.float32)
        in_maps.append(m)
    return in_maps


def kernel(**inputs):
    from concourse.bass_utils import run_bass_kernel_spmd

    x = np.asarray(inputs["x"], dtype=np.float32)
    B = x.shape[0]
    n_cores = 8
    b_loc = B // n_cores
    nc = _get_nc(n_cores, b_loc)
    in_maps = make_in_maps(inputs, n_cores)
    res = run_bass_kernel_spmd(nc, in_maps, core_ids=list(range(n_cores)))
    outs = []
    for c in range(n_cores):
        oc = np.asarray(res.results[c]["out"]).astype(np.float32)
        outs.append(oc.transpose(1, 0, 2, 3))
    return np.concatenate(outs, axis=0)
